# revision 34
# baseline (speedup 1.0000x reference)
"""GQA self-attention (B=2, t=2048, D=2048, 16 q-heads / 4 kv-heads, dk=128)
with RoPE, causal softmax, fixed-key attention dropout, output projection.

Sharding: 8 devices = batch(2) x kv-head(4). Each device computes its 4
q-heads (one kv group) for one batch over the full sequence, producing a
partial wo-projection output; the host sums the 4 partials per batch.

Orientation: scores are computed TRANSPOSED (scoresT[tk, tq]) so that the
PV matmul and the wo projection need no on-device transposes of probs.
Matmuls run as float32r (full PE rate at free-dim >= 256).
"""

import numpy as np

import concourse.bass as bass
import concourse.tile as tile
from concourse import bacc, mybir
from concourse import bass_utils
from concourse.bass import ds, ts
from concourse.masks import make_identity

F32 = mybir.dt.float32
F32R = mybir.dt.float32r
U8 = mybir.dt.uint8

B, T, D = 2, 2048, 2048
HKV, G, DK = 4, 4, 128
P = 128
CH = 512                      # tq chunk width
NCH = T // CH                 # 4 chunks
KS = D // P                   # 16 contraction subtiles for the projections
NA = T // P                   # 16 tk tiles
SCALE = 1.0 / np.sqrt(np.float32(DK))
DROP_KEEP = 0.9
NEG = -30000.0

_prog_cache = {}
_host_cache = {}


def _build_program():
    nc = bacc.Bacc("TRN2", target_bir_lowering=False, debug=False)

    xT = nc.dram_tensor("xT", [D, T], F32R, kind="ExternalInput").ap()
    wq = nc.dram_tensor("wq", [D, G * DK], F32R, kind="ExternalInput").ap()
    wk = nc.dram_tensor("wk", [D, DK], F32R, kind="ExternalInput").ap()
    wv = nc.dram_tensor("wv", [D, DK], F32R, kind="ExternalInput").ap()
    wo = nc.dram_tensor("wo", [G * DK, D], F32R, kind="ExternalInput").ap()
    cosf = nc.dram_tensor("cosf", [P, T], F32, kind="ExternalInput").ap()
    sinf = nc.dram_tensor("sinf", [P, T], F32, kind="ExternalInput").ap()
    maskT = nc.dram_tensor("maskT", [G, T, T], U8, kind="ExternalInput").ap()
    trim = nc.dram_tensor("trim", [P, 4, CH], F32R, kind="ExternalInput").ap()
    nident = nc.dram_tensor("nident", [P, P], F32R, kind="ExternalInput").ap()

    attn = nc.dram_tensor("attn", [T, D], F32, kind="ExternalOutput").ap()
    kT_out = nc.dram_tensor("kT_out", [P, T], F32R, kind="ExternalOutput").ap()
    vT_out = nc.dram_tensor("vT_out", [P, T], F32, kind="ExternalOutput").ap()

    xT_r4 = xT.rearrange("(kb k4 p) t -> p kb k4 t", p=P, k4=4)
    wq_r4 = wq.rearrange("(kb k4 p) m -> p kb k4 m", p=P, k4=4)
    wk_r4 = wk.rearrange("(kb k4 p) m -> p kb k4 m", p=P, k4=4)
    wv_r4 = wv.rearrange("(kb k4 p) m -> p kb k4 m", p=P, k4=4)
    wo_r = wo.rearrange("(g p) n -> p g n", p=P)
    attn_r = attn.rearrange("(m p) n -> p m n", p=P)
    maskT_r = maskT.rearrange("g (aa p) t -> g p aa t", p=P)

    with tile.TileContext(nc) as tc:
        with (
            tc.tile_pool(name="const", bufs=1) as const,
            tc.tile_pool(name="xp", bufs=2) as xp,
            tc.tile_pool(name="praw", bufs=3) as praw,
            tc.tile_pool(name="pmask", bufs=3) as pmask,
            tc.tile_pool(name="mkp", bufs=2) as mkp,
            tc.tile_pool(name="outp", bufs=2) as outp,
            tc.tile_pool(name="rotp", bufs=2) as rotp,
            tc.tile_pool(name="vtp", bufs=1) as vtp,
        ):
            # ---- resident SBUF tensors ----
            ones_f = const.tile([P, P], F32)
            ones_sb = const.tile([P, P], F32R)
            ident = const.tile([P, P], F32)
            qT_all = const.tile([P, G, T], F32R)   # rope applied in place
            kT_all = const.tile([P, T], F32R)      # rope applied in place
            v_sb = const.tile([P, NA, DK], F32R)   # v in [tk, v] tiles
            outT_all = const.tile([P, G, T], F32R)

            nc.vector.memset(ones_f[:], 1.0)
            nc.vector.tensor_copy(ones_sb[:], ones_f[:])
            make_identity(nc, ident[:])

            # ---- phases A (projections) + rope, weights in a closing pool --
            with (
                tc.tile_pool(name="wpool", bufs=1) as wpool,
                tc.tile_pool(name="psA", bufs=6, space="PSUM") as psA,
                tc.tile_pool(name="psAtr", bufs=1, space="PSUM") as psAtr,
            ):
                wq_sb = [wpool.tile([P, 4, G * DK], F32R, name=f"wq{kb}")
                         for kb in range(KS // 4)]
                wk_sb = [wpool.tile([P, 4, DK], F32R, name=f"wk{kb}")
                         for kb in range(KS // 4)]
                wv_sb = [wpool.tile([P, 4, DK], F32R, name=f"wv{kb}")
                         for kb in range(KS // 4)]
                cos_sb = wpool.tile([P, T], F32)
                sin_sb = wpool.tile([P, T], F32)
                for c in range(NCH):
                    q_ps = [psA.tile([P, CH], F32, tag="proj", name=f"qps{g}")
                            for g in range(G)]
                    k_ps = psA.tile([P, CH], F32, tag="proj")
                    v_ps = psA.tile([P, CH], F32, tag="proj")
                    for k in range(KS):
                        kb, k4 = divmod(k, 4)
                        if k4 == 0:
                            if c == 0:  # stream weights ahead of first use
                                nc.sync.dma_start(wq_sb[kb][:], wq_r4[:, kb])
                                nc.sync.dma_start(wk_sb[kb][:],
                                                  wk_r4[:, kb])
                                nc.sync.dma_start(wv_sb[kb][:],
                                                  wv_r4[:, kb])
                            xt4 = xp.tile([P, 4, CH], F32R, tag="xt")
                            nc.scalar.dma_start(
                                xt4[:], xT_r4[:, kb, :, ds(c * CH, CH)])
                        xt = xt4[:, k4]
                        st = (k == 0)
                        sp = (k == KS - 1)
                        for g in range(G):
                            nc.tensor.matmul(q_ps[g][:],
                                             lhsT=wq_sb[kb][:, k4, ts(g, DK)],
                                             rhs=xt, start=st, stop=sp)
                        nc.tensor.matmul(k_ps[:], lhsT=wk_sb[kb][:, k4],
                                         rhs=xt, start=st, stop=sp)
                        nc.tensor.matmul(v_ps[:], lhsT=wv_sb[kb][:, k4],
                                         rhs=xt, start=st, stop=sp)
                    for g in range(G):
                        nc.vector.tensor_copy(qT_all[:, g, ds(c * CH, CH)],
                                              q_ps[g][:])
                    nc.vector.tensor_copy(kT_all[:, ds(c * CH, CH)], k_ps[:])
                    vt = vtp.tile([P, CH], F32)
                    nc.vector.tensor_copy(vt[:], v_ps[:])
                    nc.sync.dma_start(vT_out[:, ds(c * CH, CH)], vt[:])
                    # transpose vT chunk -> v_sb tiles [tk, v]
                    for j in range(CH // P):
                        a = c * (CH // P) + j
                        tp = psAtr.tile([P, P], F32)
                        nc.tensor.transpose(tp[:], vt[:, ts(j, P)], ident[:])
                        nc.scalar.copy(v_sb[:, a], tp[:])

                    # rope in batches, in place (cos/sin loaded with c==0)
                    if c == 0:
                        nc.sync.dma_start(cos_sb[:], cosf)
                        nc.sync.dma_start(sin_sb[:], sinf)

                    def rope(dst, lo, width, tag):
                        sl = ds(lo, width)
                        rot = rotp.tile([P, CH], F32R, tag="rot", name=tag)
                        nc.sync.dma_start(rot[:64, :width], dst[64:, sl])
                        nc.sync.dma_start(rot[64:, :width], dst[:64, sl])
                        nc.vector.tensor_mul(dst[:, sl], dst[:, sl],
                                             cos_sb[:, sl])
                        nc.vector.tensor_mul(rot[:, :width], rot[:, :width],
                                             sin_sb[:, sl])
                        nc.vector.tensor_add(dst[:, sl], dst[:, sl],
                                             rot[:, :width])

                    rope(kT_all, c * CH, CH, f"ropek{c}")
                    nc.sync.dma_start(kT_out[:, ds(c * CH, CH)],
                                      kT_all[:, ds(c * CH, CH)])
                    for g in range(G):
                        rope(qT_all[:, g], c * CH, CH, f"ropeq{g}c{c}")

            # ---- phases B (attention) + C (wo projection), interleaved ----
            # B is c-outer / g-inner; once chunk c is done for all g, the
            # wo-projection rows for that chunk (m-tiles 4c..4c+3) run.
            # Causal zeros live in maskT (host-side AND); crossing-diagonal
            # tiles get a causal-only copy (prz, GpSimd affine_select) for
            # the pre-dropout rowsum. Emission is software-pipelined by LAG.
            LAG = 2
            with (
                tc.tile_pool(name="psB", bufs=3, space="PSUM") as psB,
                tc.tile_pool(name="psBrs", bufs=2, space="PSUM") as psBrs,
                tc.tile_pool(name="psBpv", bufs=2, space="PSUM") as psBpv,
                tc.tile_pool(name="psC", bufs=1, space="PSUM") as psC,
                tc.tile_pool(name="wosp", bufs=1) as wosp,
            ):
                wo_sb = wosp.tile([P, G, T], F32R)  # reuses freed wpool space
                trim_sb = wosp.tile([P, 4, CH], F32R)
                nident_sb = wosp.tile([P, P], F32R)
                nc.sync.dma_start(nident_sb[:], nident)
                nc.sync.dma_start(trim_sb[:], trim)
                # wo is needed only once chunk 0 completes; keep it behind
                # trim/nident and off the mask (ACT) ring
                nc.sync.dma_start(wo_sb[:], wo_r)
                state = {}

                def sc_stage(g, c, a):
                    if (g, c) not in state:
                        n_a = 4 * c + 4
                        mk = mkp.tile([P, NA, CH], U8, tag="mk",
                                      name=f"mk{g}_{c}")
                        nc.scalar.dma_start(
                            mk[:, :n_a],
                            maskT_r[g, :, :n_a, ds(c * CH, CH)])
                        state[(g, c)] = {
                            "rs": psBrs.tile([P, CH], F32, tag="rs",
                                             name=f"rs{g}_{c}"),
                            "pv": psBpv.tile([P, CH], F32, tag="pv",
                                             name=f"pv{g}_{c}"),
                            "pr": {},
                            "mk": mk,
                        }
                    st = state[(g, c)]
                    s_ps = psB.tile([P, CH], F32, tag="sc", name=f"sps{a}")
                    r = a - 4 * c
                    nc.tensor.matmul(
                        s_ps[:], lhsT=kT_all[:, ts(a, P)],
                        rhs=qT_all[:, g, ds(c * CH, CH)],
                        start=True, stop=(r < 0))
                    if r >= 0:  # crossing tile: += NEG * tri[r] on PE
                        nc.tensor.matmul(
                            s_ps[:], lhsT=nident_sb[:], rhs=trim_sb[:, r],
                            start=False, stop=True)
                    pr = praw.tile([P, CH], F32R, tag="praw", name=f"pr{a}")
                    nc.scalar.activation(
                        pr[:], s_ps[:], mybir.ActivationFunctionType.Exp,
                        scale=float(SCALE))
                    st["pr"][a] = pr

                def tail_stage(g, c, a):
                    st = state[(g, c)]
                    n_a = 4 * c + 4
                    pr = st["pr"].pop(a)
                    pm = pmask.tile([P, CH], F32R, tag="pm", name=f"pm{a}")
                    if a % 2 == 0:
                        nc.vector.tensor_mul(pm[:], pr[:], st["mk"][:, a])
                    else:
                        nc.gpsimd.tensor_mul(pm[:], pr[:], st["mk"][:, a])
                    nc.tensor.matmul(st["rs"][:], lhsT=ones_sb[:],
                                     rhs=pr[:],
                                     start=(a == 0), stop=(a == n_a - 1))
                    nc.tensor.matmul(st["pv"][:], lhsT=v_sb[:, a],
                                     rhs=pm[:],
                                     start=(a == 0), stop=(a == n_a - 1))
                    if a == n_a - 1:  # epilogue: normalize into outT_all
                        rec = rotp.tile([P, CH], F32, tag="rec")
                        nc.vector.reciprocal(rec[:], st["rs"][:])
                        nc.vector.scalar_tensor_tensor(
                            outT_all[:, g, ds(c * CH, CH)], st["pv"][:],
                            1.0 / DROP_KEEP, rec[:],
                            mybir.AluOpType.mult, mybir.AluOpType.mult)
                        del state[(g, c)]

                def wo_rows(m):
                    for half in range(2):
                        ob = outp.tile([P, 2, CH], F32, tag="ot",
                                       name=f"ob{m}_{half}")
                        for i in range(2):
                            c2 = half * 2 + i
                            op = psC.tile([P, CH], F32, tag="wops")
                            for g in range(G):
                                nc.tensor.matmul(
                                    op[:], lhsT=outT_all[:, g, ts(m, P)],
                                    rhs=wo_sb[:, g, ds(c2 * CH, CH)],
                                    start=(g == 0), stop=(g == G - 1))
                            if c2 % 2 == 0:
                                nc.scalar.copy(ob[:, i], op[:])
                            else:
                                nc.vector.tensor_copy(ob[:, i], op[:])
                        nc.sync.dma_start(
                            attn_r[:, m, ds(half * 2 * CH, 2 * CH)], ob[:])

                LAG = 2
                tasks = []
                for c in range(NCH):
                    for g0, g1 in [(0, 1), (2, 3)]:
                        for a in range(4 * c + 4):
                            tasks.append((g0, c, a))
                            tasks.append((g1, c, a))
                # wo_rows(m) for chunk c become ready when all g of chunk c
                # are done; emit them as soon as the pipeline crosses that
                # point so PE has independent work at chunk boundaries.
                n_tasks = len(tasks)
                done_upto = {}
                pos = 0
                for c in range(NCH):
                    pos += G * (4 * c + 4)
                    done_upto[pos] = c
                emitted_c = set()
                for i in range(n_tasks + LAG):
                    if i < n_tasks:
                        sc_stage(*tasks[i])
                    j = i - LAG
                    if j >= 0:
                        tail_stage(*tasks[j])
                        if j + 1 in done_upto:
                            c_done = done_upto[j + 1]
                            if c_done not in emitted_c:
                                emitted_c.add(c_done)
                                for m in range(4 * c_done, 4 * c_done + 4):
                                    wo_rows(m)

    nc.compile()
    return nc


def _host_inputs(x, wq, wk, wv, wo):
    """Per-device input dicts. Cached pieces that don't depend on inputs."""
    if "mask" not in _host_cache:
        import jax
        import jax.numpy as jnp
        # Mask bits MUST come from the same backend the reference runs on
        # (threefry bit-streams differ between cpu and neuron backends here).
        keep = jax.random.bernoulli(
            jax.random.key(1), DROP_KEEP, (B, G, HKV, T, T))
        keep = np.asarray(keep)  # bool [B, G, H, tq, tk]
        # maskT[dev b,h] = keep[b, :, h].transpose(0, 2, 1) as u8
        causal_T = (np.arange(T)[:, None] <= np.arange(T)[None, :])
        _host_cache["mask"] = [
            (np.ascontiguousarray(keep[b, :, h].transpose(0, 2, 1))
             & causal_T[None]).astype(np.uint8)
            for b in range(B) for h in range(HKV)]
        theta = jnp.float32(10000.0) ** (
            -jnp.arange(0, DK, 2, dtype=jnp.float32) / DK)
        ticks = jnp.outer(jnp.arange(T, dtype=jnp.float32), theta)
        cos = np.asarray(jnp.cos(ticks)).T      # [64, T]
        sin = np.asarray(jnp.sin(ticks)).T
        _host_cache["cos"] = np.concatenate([cos, cos], 0).astype(np.float32)
        _host_cache["sin"] = np.concatenate([-sin, sin], 0).astype(np.float32)

    if "trim" not in _host_cache:
        p = np.arange(P)[:, None]
        f = np.arange(CH)[None, :]
        tri = np.zeros((P, 4, CH), np.float32)
        for r in range(4):
            tri[:, r, :] = (p > f - P * r).astype(np.float32)
        _host_cache["trim"] = tri
        _host_cache["nident"] = (NEG * np.eye(P)).astype(np.float32)

    in_maps = []
    for dev in range(8):
        b, h = dev // HKV, dev % HKV
        wq_dev = np.ascontiguousarray(np.concatenate(
            [wq[:, (g * HKV + h) * DK:(g * HKV + h + 1) * DK] for g in range(G)],
            axis=1))
        wo_dev = np.ascontiguousarray(np.concatenate(
            [wo[g * (HKV * DK) + h * DK: g * (HKV * DK) + (h + 1) * DK]
             for g in range(G)], axis=0))
        in_maps.append({
            "xT": np.ascontiguousarray(x[b].T),
            "wq": wq_dev,
            "wk": np.ascontiguousarray(wk[:, h * DK:(h + 1) * DK]),
            "wv": np.ascontiguousarray(wv[:, h * DK:(h + 1) * DK]),
            "wo": wo_dev,
            "cosf": _host_cache["cos"],
            "sinf": _host_cache["sin"],
            "maskT": _host_cache["mask"][dev],
            "trim": _host_cache["trim"],
            "nident": _host_cache["nident"],
        })
    return in_maps


def _run(in_maps, **kw):
    if "nc" not in _prog_cache:
        _prog_cache["nc"] = _build_program()
    return bass_utils.run_bass_kernel_spmd(
        _prog_cache["nc"], in_maps, core_ids=list(range(8)), **kw)


def kernel(x, wq, wk, wv, wo, _results_out=None, **run_kw):
    x = np.asarray(x); wq = np.asarray(wq); wk = np.asarray(wk)
    wv = np.asarray(wv); wo = np.asarray(wo)
    res = _run(_host_inputs(x, wq, wk, wv, wo), **run_kw)
    outs = [{k: np.asarray(v) for k, v in r.items()} for r in res.results]
    if _results_out is not None:
        _results_out.append(res)
    attn = np.stack(
        [sum(outs[b * HKV + h]["attn"] for h in range(HKV)) for b in range(B)])
    kv = np.empty((B, T, HKV, 2 * DK), np.float32)
    for dev in range(8):
        b, h = dev // HKV, dev % HKV
        kv[b, :, h, :DK] = outs[dev]["kT_out"].T
        kv[b, :, h, DK:] = outs[dev]["vT_out"].T
    return attn, kv


# revision 35
# speedup vs baseline: 1.1145x; 1.1145x over previous
"""GQA self-attention (B=2, t=2048, D=2048, 16 q-heads / 4 kv-heads, dk=128)
with RoPE, causal softmax, fixed-key attention dropout, output projection.

Sharding: 8 devices = batch(2) x kv-head(4). Each device computes its 4
q-heads (one kv group) for one batch over the full sequence, producing a
partial wo-projection output; the host sums the 4 partials per batch.

Orientation: scores are computed TRANSPOSED (scoresT[tk, tq]) so that the
PV matmul and the wo projection need no on-device transposes of probs.
Matmuls run as float32r (full PE rate at free-dim >= 256).
"""

import numpy as np

import concourse.bass as bass
import concourse.tile as tile
from concourse import bacc, mybir
from concourse import bass_utils
from concourse.bass import ds, ts
from concourse.masks import make_identity

F32 = mybir.dt.float32
F32R = mybir.dt.float32r
U8 = mybir.dt.uint8

B, T, D = 2, 2048, 2048
HKV, G, DK = 4, 4, 128
P = 128
CH = 512                      # tq chunk width
NCH = T // CH                 # 4 chunks
KS = D // P                   # 16 contraction subtiles for the projections
NA = T // P                   # 16 tk tiles
SCALE = 1.0 / np.sqrt(np.float32(DK))
DROP_KEEP = 0.9
NEG = -30000.0

_prog_cache = {}
_host_cache = {}


def _build_program():
    nc = bacc.Bacc("TRN2", target_bir_lowering=False, debug=False)

    xT = nc.dram_tensor("xT", [D, T], F32R, kind="ExternalInput").ap()
    wq = nc.dram_tensor("wq", [D, G * DK], F32R, kind="ExternalInput").ap()
    wk = nc.dram_tensor("wk", [D, DK], F32R, kind="ExternalInput").ap()
    wv = nc.dram_tensor("wv", [D, DK], F32R, kind="ExternalInput").ap()
    wo = nc.dram_tensor("wo", [G * DK, D], F32R, kind="ExternalInput").ap()
    cosf = nc.dram_tensor("cosf", [P, T], F32, kind="ExternalInput").ap()
    sinf = nc.dram_tensor("sinf", [P, T], F32, kind="ExternalInput").ap()
    maskT = nc.dram_tensor("maskT", [G, T, T], U8, kind="ExternalInput").ap()
    trim = nc.dram_tensor("trim", [P, 4, CH], F32R, kind="ExternalInput").ap()
    nident = nc.dram_tensor("nident", [P, P], F32R, kind="ExternalInput").ap()

    attn = nc.dram_tensor("attn", [T, D], F32, kind="ExternalOutput").ap()
    kT_out = nc.dram_tensor("kT_out", [P, T], F32R, kind="ExternalOutput").ap()
    vT_out = nc.dram_tensor("vT_out", [P, T], F32, kind="ExternalOutput").ap()

    xT_r4 = xT.rearrange("(kb k4 p) t -> p kb k4 t", p=P, k4=4)
    wq_r4 = wq.rearrange("(kb k4 p) m -> p kb k4 m", p=P, k4=4)
    wk_r4 = wk.rearrange("(kb k4 p) m -> p kb k4 m", p=P, k4=4)
    wv_r4 = wv.rearrange("(kb k4 p) m -> p kb k4 m", p=P, k4=4)
    wo_r = wo.rearrange("(g p) n -> p g n", p=P)
    attn_r = attn.rearrange("(m p) n -> p m n", p=P)
    maskT_r = maskT.rearrange("g (aa p) t -> g p aa t", p=P)

    with tile.TileContext(nc) as tc:
        with (
            tc.tile_pool(name="const", bufs=1) as const,
            tc.tile_pool(name="xp", bufs=2) as xp,
            tc.tile_pool(name="praw", bufs=3) as praw,
            tc.tile_pool(name="pmask", bufs=3) as pmask,
            tc.tile_pool(name="mkp", bufs=2) as mkp,
            tc.tile_pool(name="outp", bufs=2) as outp,
            tc.tile_pool(name="rotp", bufs=2) as rotp,
            tc.tile_pool(name="vtp", bufs=1) as vtp,
        ):
            # ---- resident SBUF tensors ----
            ones_f = const.tile([P, P], F32)
            ones_sb = const.tile([P, P], F32R)
            ident = const.tile([P, P], F32)
            qT_all = const.tile([P, G, T], F32R)   # rope applied in place
            kT_all = const.tile([P, T], F32R)      # rope applied in place
            v_sb = const.tile([P, NA, DK], F32R)   # v in [tk, v] tiles
            outT_all = const.tile([P, G, T], F32R)

            nc.vector.memset(ones_f[:], 1.0)
            nc.vector.tensor_copy(ones_sb[:], ones_f[:])
            make_identity(nc, ident[:])

            # ---- phases A (projections) + rope, weights in a closing pool --
            with (
                tc.tile_pool(name="wpool", bufs=1) as wpool,
                tc.tile_pool(name="psA", bufs=7, space="PSUM") as psA,
                tc.tile_pool(name="psAtr", bufs=1, space="PSUM") as psAtr,
            ):
                wq_sb = [wpool.tile([P, 4, G * DK], F32R, name=f"wq{kb}")
                         for kb in range(KS // 4)]
                wk_sb = [wpool.tile([P, 4, DK], F32R, name=f"wk{kb}")
                         for kb in range(KS // 4)]
                wv_sb = [wpool.tile([P, 4, DK], F32R, name=f"wv{kb}")
                         for kb in range(KS // 4)]
                cos_sb = wpool.tile([P, T], F32)
                sin_sb = wpool.tile([P, T], F32)
                for c in range(NCH):
                    q_ps = [psA.tile([P, CH], F32, tag="proj", name=f"qps{g}")
                            for g in range(G)]
                    k_ps = psA.tile([P, CH], F32, tag="proj")
                    v_ps = psA.tile([P, CH], F32, tag="proj")
                    for k in range(KS):
                        kb, k4 = divmod(k, 4)
                        if k4 == 0:
                            if c == 0:  # stream weights ahead of first use
                                nc.sync.dma_start(wq_sb[kb][:], wq_r4[:, kb])
                                nc.sync.dma_start(wk_sb[kb][:],
                                                  wk_r4[:, kb])
                                nc.sync.dma_start(wv_sb[kb][:],
                                                  wv_r4[:, kb])
                            xt4 = xp.tile([P, 4, CH], F32R, tag="xt")
                            nc.scalar.dma_start(
                                xt4[:], xT_r4[:, kb, :, ds(c * CH, CH)])
                        xt = xt4[:, k4]
                        st = (k == 0)
                        sp = (k == KS - 1)
                        for g in range(G):
                            nc.tensor.matmul(q_ps[g][:],
                                             lhsT=wq_sb[kb][:, k4, ts(g, DK)],
                                             rhs=xt, start=st, stop=sp)
                        nc.tensor.matmul(k_ps[:], lhsT=wk_sb[kb][:, k4],
                                         rhs=xt, start=st, stop=sp)
                        nc.tensor.matmul(v_ps[:], lhsT=wv_sb[kb][:, k4],
                                         rhs=xt, start=st, stop=sp)
                    for g in range(G):
                        nc.scalar.copy(qT_all[:, g, ds(c * CH, CH)],
                                       q_ps[g][:])
                    nc.vector.tensor_copy(kT_all[:, ds(c * CH, CH)], k_ps[:])
                    vt = vtp.tile([P, CH], F32)
                    nc.vector.tensor_copy(vt[:], v_ps[:])
                    nc.sync.dma_start(vT_out[:, ds(c * CH, CH)], vt[:])
                    # transpose vT chunk -> v_sb tiles [tk, v]
                    for j in range(CH // P):
                        a = c * (CH // P) + j
                        tp = psAtr.tile([P, P], F32)
                        nc.tensor.transpose(tp[:], vt[:, ts(j, P)], ident[:])
                        nc.scalar.copy(v_sb[:, a], tp[:])

                    # rope in batches, in place (cos/sin loaded with c==0)
                    if c == 0:
                        nc.sync.dma_start(cos_sb[:], cosf)
                        nc.sync.dma_start(sin_sb[:], sinf)

                    def rope(dst, lo, width, tag):
                        sl = ds(lo, width)
                        rot = rotp.tile([P, CH], F32R, tag="rot", name=tag)
                        nc.sync.dma_start(rot[:64, :width], dst[64:, sl])
                        nc.sync.dma_start(rot[64:, :width], dst[:64, sl])
                        nc.vector.tensor_mul(dst[:, sl], dst[:, sl],
                                             cos_sb[:, sl])
                        nc.vector.tensor_mul(rot[:, :width], rot[:, :width],
                                             sin_sb[:, sl])
                        nc.vector.tensor_add(dst[:, sl], dst[:, sl],
                                             rot[:, :width])

                    rope(kT_all, c * CH, CH, f"ropek{c}")
                    nc.sync.dma_start(kT_out[:, ds(c * CH, CH)],
                                      kT_all[:, ds(c * CH, CH)])
                    for g in range(G):
                        rope(qT_all[:, g], c * CH, CH, f"ropeq{g}c{c}")

            # ---- phases B (attention) + C (wo projection), interleaved ----
            # B is c-outer / g-inner; once chunk c is done for all g, the
            # wo-projection rows for that chunk (m-tiles 4c..4c+3) run.
            # Causal zeros live in maskT (host-side AND); crossing-diagonal
            # tiles get a causal-only copy (prz, GpSimd affine_select) for
            # the pre-dropout rowsum. Emission is software-pipelined by LAG.
            LAG = 2
            with (
                tc.tile_pool(name="psB", bufs=3, space="PSUM") as psB,
                tc.tile_pool(name="psBrs", bufs=2, space="PSUM") as psBrs,
                tc.tile_pool(name="psBpv", bufs=2, space="PSUM") as psBpv,
                tc.tile_pool(name="psC", bufs=1, space="PSUM") as psC,
                tc.tile_pool(name="wosp", bufs=1) as wosp,
            ):
                wo_sb = wosp.tile([P, G, T], F32R)  # reuses freed wpool space
                trim_sb = wosp.tile([P, 4, CH], F32R)
                nident_sb = wosp.tile([P, P], F32R)
                nc.sync.dma_start(nident_sb[:], nident)
                nc.sync.dma_start(trim_sb[:], trim)
                # wo is needed only once chunk 0 completes; keep it behind
                # trim/nident and off the mask (ACT) ring
                nc.sync.dma_start(wo_sb[:], wo_r)
                state = {}

                def sc_stage(g, c, a):
                    if (g, c) not in state:
                        n_a = 4 * c + 4
                        mk = mkp.tile([P, NA, CH], U8, tag="mk",
                                      name=f"mk{g}_{c}")
                        nc.scalar.dma_start(
                            mk[:, :n_a],
                            maskT_r[g, :, :n_a, ds(c * CH, CH)])
                        state[(g, c)] = {
                            "rs": psBrs.tile([P, CH], F32, tag="rs",
                                             name=f"rs{g}_{c}"),
                            "pv": psBpv.tile([P, CH], F32, tag="pv",
                                             name=f"pv{g}_{c}"),
                            "pr": {},
                            "mk": mk,
                        }
                    st = state[(g, c)]
                    s_ps = psB.tile([P, CH], F32, tag="sc", name=f"sps{a}")
                    r = a - 4 * c
                    nc.tensor.matmul(
                        s_ps[:], lhsT=kT_all[:, ts(a, P)],
                        rhs=qT_all[:, g, ds(c * CH, CH)],
                        start=True, stop=(r < 0))
                    if r >= 0:  # crossing tile: += NEG * tri[r] on PE
                        nc.tensor.matmul(
                            s_ps[:], lhsT=nident_sb[:], rhs=trim_sb[:, r],
                            start=False, stop=True)
                    pr = praw.tile([P, CH], F32R, tag="praw", name=f"pr{a}")
                    nc.scalar.activation(
                        pr[:], s_ps[:], mybir.ActivationFunctionType.Exp,
                        scale=float(SCALE))
                    st["pr"][a] = pr

                def tail_stage(g, c, a):
                    st = state[(g, c)]
                    n_a = 4 * c + 4
                    pr = st["pr"].pop(a)
                    pm = pmask.tile([P, CH], F32R, tag="pm", name=f"pm{a}")
                    if a % 2 == 0:
                        nc.vector.tensor_mul(pm[:], pr[:], st["mk"][:, a])
                    else:
                        nc.gpsimd.tensor_mul(pm[:], pr[:], st["mk"][:, a])
                    nc.tensor.matmul(st["rs"][:], lhsT=ones_sb[:],
                                     rhs=pr[:],
                                     start=(a == 0), stop=(a == n_a - 1))
                    nc.tensor.matmul(st["pv"][:], lhsT=v_sb[:, a],
                                     rhs=pm[:],
                                     start=(a == 0), stop=(a == n_a - 1))
                    if a == n_a - 1:  # epilogue: normalize into outT_all
                        rec = rotp.tile([P, CH], F32, tag="rec")
                        nc.vector.reciprocal(rec[:], st["rs"][:])
                        nc.vector.scalar_tensor_tensor(
                            outT_all[:, g, ds(c * CH, CH)], st["pv"][:],
                            1.0 / DROP_KEEP, rec[:],
                            mybir.AluOpType.mult, mybir.AluOpType.mult)
                        del state[(g, c)]

                def wo_rows(m):
                    for half in range(2):
                        ob = outp.tile([P, 2, CH], F32, tag="ot",
                                       name=f"ob{m}_{half}")
                        for i in range(2):
                            c2 = half * 2 + i
                            op = psC.tile([P, CH], F32, tag="wops")
                            for g in range(G):
                                nc.tensor.matmul(
                                    op[:], lhsT=outT_all[:, g, ts(m, P)],
                                    rhs=wo_sb[:, g, ds(c2 * CH, CH)],
                                    start=(g == 0), stop=(g == G - 1))
                            if c2 % 2 == 0:
                                nc.scalar.copy(ob[:, i], op[:])
                            else:
                                nc.vector.tensor_copy(ob[:, i], op[:])
                        nc.sync.dma_start(
                            attn_r[:, m, ds(half * 2 * CH, 2 * CH)], ob[:])

                LAG = 2
                tasks = []
                for c in range(NCH):
                    for g0, g1 in [(0, 1), (2, 3)]:
                        for a in range(4 * c + 4):
                            tasks.append((g0, c, a))
                            tasks.append((g1, c, a))
                # wo_rows(m) for chunk c become ready when all g of chunk c
                # are done; emit them as soon as the pipeline crosses that
                # point so PE has independent work at chunk boundaries.
                n_tasks = len(tasks)
                done_upto = {}
                pos = 0
                for c in range(NCH):
                    pos += G * (4 * c + 4)
                    done_upto[pos] = c
                emitted_c = set()
                for i in range(n_tasks + LAG):
                    if i < n_tasks:
                        sc_stage(*tasks[i])
                    j = i - LAG
                    if j >= 0:
                        tail_stage(*tasks[j])
                        if j + 1 in done_upto:
                            c_done = done_upto[j + 1]
                            if c_done not in emitted_c:
                                emitted_c.add(c_done)
                                for m in range(4 * c_done, 4 * c_done + 4):
                                    wo_rows(m)

    nc.compile()
    return nc


def _host_inputs(x, wq, wk, wv, wo):
    """Per-device input dicts. Cached pieces that don't depend on inputs."""
    if "mask" not in _host_cache:
        import jax
        import jax.numpy as jnp
        # Mask bits MUST come from the same backend the reference runs on
        # (threefry bit-streams differ between cpu and neuron backends here).
        keep = jax.random.bernoulli(
            jax.random.key(1), DROP_KEEP, (B, G, HKV, T, T))
        keep = np.asarray(keep)  # bool [B, G, H, tq, tk]
        # maskT[dev b,h] = keep[b, :, h].transpose(0, 2, 1) as u8
        causal_T = (np.arange(T)[:, None] <= np.arange(T)[None, :])
        _host_cache["mask"] = [
            (np.ascontiguousarray(keep[b, :, h].transpose(0, 2, 1))
             & causal_T[None]).astype(np.uint8)
            for b in range(B) for h in range(HKV)]
        theta = jnp.float32(10000.0) ** (
            -jnp.arange(0, DK, 2, dtype=jnp.float32) / DK)
        ticks = jnp.outer(jnp.arange(T, dtype=jnp.float32), theta)
        cos = np.asarray(jnp.cos(ticks)).T      # [64, T]
        sin = np.asarray(jnp.sin(ticks)).T
        _host_cache["cos"] = np.concatenate([cos, cos], 0).astype(np.float32)
        _host_cache["sin"] = np.concatenate([-sin, sin], 0).astype(np.float32)

    if "trim" not in _host_cache:
        p = np.arange(P)[:, None]
        f = np.arange(CH)[None, :]
        tri = np.zeros((P, 4, CH), np.float32)
        for r in range(4):
            tri[:, r, :] = (p > f - P * r).astype(np.float32)
        _host_cache["trim"] = tri
        _host_cache["nident"] = (NEG * np.eye(P)).astype(np.float32)

    in_maps = []
    for dev in range(8):
        b, h = dev // HKV, dev % HKV
        wq_dev = np.ascontiguousarray(np.concatenate(
            [wq[:, (g * HKV + h) * DK:(g * HKV + h + 1) * DK] for g in range(G)],
            axis=1))
        wo_dev = np.ascontiguousarray(np.concatenate(
            [wo[g * (HKV * DK) + h * DK: g * (HKV * DK) + (h + 1) * DK]
             for g in range(G)], axis=0))
        in_maps.append({
            "xT": np.ascontiguousarray(x[b].T),
            "wq": wq_dev,
            "wk": np.ascontiguousarray(wk[:, h * DK:(h + 1) * DK]),
            "wv": np.ascontiguousarray(wv[:, h * DK:(h + 1) * DK]),
            "wo": wo_dev,
            "cosf": _host_cache["cos"],
            "sinf": _host_cache["sin"],
            "maskT": _host_cache["mask"][dev],
            "trim": _host_cache["trim"],
            "nident": _host_cache["nident"],
        })
    return in_maps


def _run(in_maps, **kw):
    if "nc" not in _prog_cache:
        _prog_cache["nc"] = _build_program()
    return bass_utils.run_bass_kernel_spmd(
        _prog_cache["nc"], in_maps, core_ids=list(range(8)), **kw)


def kernel(x, wq, wk, wv, wo, _results_out=None, **run_kw):
    x = np.asarray(x); wq = np.asarray(wq); wk = np.asarray(wk)
    wv = np.asarray(wv); wo = np.asarray(wo)
    res = _run(_host_inputs(x, wq, wk, wv, wo), **run_kw)
    outs = [{k: np.asarray(v) for k, v in r.items()} for r in res.results]
    if _results_out is not None:
        _results_out.append(res)
    attn = np.stack(
        [sum(outs[b * HKV + h]["attn"] for h in range(HKV)) for b in range(B)])
    kv = np.empty((B, T, HKV, 2 * DK), np.float32)
    for dev in range(8):
        b, h = dev // HKV, dev % HKV
        kv[b, :, h, :DK] = outs[dev]["kT_out"].T
        kv[b, :, h, DK:] = outs[dev]["vT_out"].T
    return attn, kv
